# revision 12
# baseline (speedup 1.0000x reference)
"""Block-sparse flash attention (local + vertical-stride pattern) on 8 TRN2
NeuronCores.

Sharding: tensor-parallel over heads. Core c gets q-heads [4c, 4c+4) and
kv-head c (the GQA group maps exactly: q-head h uses kv-head h//4). No
collectives needed; outputs are concatenated along the feature dim on host.

Per-core kernel (all shapes static, fully unrolled):
  - q is processed in tiles of 256 rows (4 sparse blocks of 64).
  - Scores are computed transposed, S^T[kv, q], per 128-wide kv chunk:
      matmul(out=S^T chunk, lhsT=K^T[:, kv_chunk], rhs=Q^T[:, q_tile])
    so softmax never needs a transpose of P for the PV matmul.
  - exp (with the 1/sqrt(d) scale folded in) runs on the scalar engine
    straight out of PSUM into SBUF bf16, one instruction per PSUM group
    tile (6 kv chunks) so score matmuls of the next tile can reuse PSUM
    while exp of the previous group is still running.
  - Masking is multiplicative on P^T after exp: a combined causal mask for
    the two diagonal chunks, a per-(head,tile) combined mask for the window
    leading edge ("staircase", host-supplied data so the SPMD program is
    core-independent), and 0/1 selectors for remote-block candidates.
  - PV accumulates O[q, d] with lhsT=P^T chunk and rhs=[V | 1]; the ones
    column makes the softmax denominator fall out of the same matmuls.
  - Emission is software-pipelined: tile t's score matmuls are emitted
    before tile t-1's PV matmuls, so the (in-order) tensor engine never
    waits on the exp of the tile it is about to consume.
"""

import numpy as np
import ml_dtypes

BF16 = ml_dtypes.bfloat16

# Problem constants (hardcoded; see module docstring).
S = 2048
NUM_HEADS = 32
NUM_KV_HEADS = 8
D = 128
BLK = 64
LOCAL_BLOCKS = 16
VSTRIDE = 8
SCALE = 0.08838834764831845
NCORES = 8
HPC = NUM_HEADS // NCORES          # heads per core = 4
QTILE = 256                        # q rows per tile (4 sparse blocks)
NT = S // QTILE                    # 8 tiles
NCHUNK = S // 128                  # 16 kv chunks of 128
GROUP = 6                          # kv chunks per PSUM group tile (3 banks)


def _tile_plan(j, t):
    """Static slot plan for head-slot j (0..3), q-tile t. Core-independent.

    Returns (lo, loc, cands):
      lo    - first block of the local window (after padding to chunk align)
      loc   - [(b0, b1)] block pairs forming the local 128-wide kv chunks
      cands - candidate remote blocks (union over both core parities)
    """
    lo = max(0, 4 * t - 16)
    hi = 4 * t + 3
    loc = [(lo + 2 * i, lo + 2 * i + 1) for i in range((hi - lo + 1) // 2)]
    cands = [b for b in range(lo) if b % 4 == (3 - j) % 4]
    return lo, loc, cands


def _remote_class(core, j):
    """Blocks b with b % 8 == this value are remote-visible for head 4*core+j."""
    return (-(4 * core + j + 1)) % VSTRIDE


_CACHE = {}


def _build_nc():
    import concourse.bacc as bacc
    import concourse.tile as tile
    from concourse import mybir

    dt = mybir.dt
    nc = bacc.Bacc(None)

    qT = nc.declare_dram_parameter("qT", [HPC * D, S], dt.bfloat16, isOutput=False)
    kT = nc.declare_dram_parameter("kT", [D, S], dt.bfloat16, isOutput=False)
    v1 = nc.declare_dram_parameter("v1", [D, NCHUNK * 129], dt.bfloat16, isOutput=False)
    m1 = nc.declare_dram_parameter("m1", [D, QTILE], dt.bfloat16, isOutput=False)
    smask = nc.declare_dram_parameter("smask", [D, HPC * 4 * 512], dt.bfloat16, isOutput=False)
    rsel = nc.declare_dram_parameter("rsel", [D, HPC * VSTRIDE], dt.float32, isOutput=False)
    out = nc.declare_dram_parameter("out", [S, HPC * D], dt.float32, isOutput=True)

    EXP = mybir.ActivationFunctionType.Exp

    with tile.TileContext(nc) as tc:
        with (
            tc.tile_pool(name="consts", bufs=1) as consts,
            tc.tile_pool(name="ptp", bufs=3) as ptp,
            tc.tile_pool(name="ohp", bufs=2) as ohp,
            tc.tile_pool(name="lp", bufs=4) as lp,
            tc.tile_pool(name="stp", bufs=2, space="PSUM") as stp,
            tc.tile_pool(name="opp", bufs=2, space="PSUM") as opp,
        ):
            # warm the ACT exp table while input DMAs are in flight
            DUMI = consts.tile([128, 1], dt.float32, tag="dumi")
            DUMO = consts.tile([128, 1], dt.bfloat16, tag="dumo")
            nc.vector.memset(DUMI, 0.0)
            nc.scalar.activation(DUMO, DUMI, EXP, scale=1.0)

            # issue order matters: the first tiles need KT and QT0 only
            KT = consts.tile([128, S], dt.bfloat16, tag="kt")
            nc.sync.dma_start(out=KT[:, 0:256], in_=kT[:, 0:256])
            QT = [consts.tile([128, S], dt.bfloat16, name=f"qt{h}", tag=f"qt{h}") for h in range(HPC)]
            nc.sync.dma_start(out=QT[0][:, 0:256], in_=qT[0:128, 0:256])
            nc.sync.dma_start(out=KT[:, 256:1024], in_=kT[:, 256:1024])
            nc.sync.dma_start(out=QT[0][:, 256:1024], in_=qT[0:128, 256:1024])
            nc.sync.dma_start(out=KT[:, 1024:S], in_=kT[:, 1024:S])
            nc.sync.dma_start(out=QT[0][:, 1024:S], in_=qT[0:128, 1024:S])
            V1 = consts.tile([128, NCHUNK * 129], dt.bfloat16, tag="v1")
            nc.sync.dma_start(out=V1, in_=v1[:, :])
            M1 = consts.tile([128, QTILE], dt.bfloat16, tag="m1")
            nc.sync.dma_start(out=M1, in_=m1[:, :])
            RSEL = consts.tile([128, HPC * VSTRIDE], dt.float32, tag="rsel")
            nc.sync.dma_start(out=RSEL, in_=rsel[:, :])
            for h in range(1, HPC):
                nc.sync.dma_start(out=QT[h], in_=qT[h * 128:(h + 1) * 128, :])
            SMASK = consts.tile([128, HPC * 4 * 512], dt.bfloat16, tag="smask")
            nc.sync.dma_start(out=SMASK, in_=smask[:, :])

            def emit_scores(h, t, OH):
                """Score matmuls + exp + masks for tile (h, t). Returns the
                context needed later for its PV + epilogue.

                Slot layout: local chunks first (the last local chunk "D1" --
                diagonal blocks 4t+2,4t+3 -- is only 128 q columns wide since
                its first q-half is fully above the causal diagonal), then
                remote candidates. Slots pack greedily into 3-bank PSUM group
                tiles, one exp instruction per group."""
                lo, loc, cands = _tile_plan(h, t)
                nloc = len(loc)
                qs = QT[h][:, t * QTILE:(t + 1) * QTILE]

                # slot table: (kind, idx, col, width). The half-width D1
                # slot goes last so every other slot stays 256-aligned and no
                # matmul output straddles a PSUM bank boundary.
                slots = []
                col = 0
                for i in range(nloc - 1):
                    slots.append(("loc", i, col, QTILE))
                    col += QTILE
                for ri in range(len(cands)):
                    slots.append(("rem", ri, col, QTILE))
                    col += QTILE
                slots.append(("loc", nloc - 1, col, 128))
                col += 128

                PT = ptp.tile([128, 10 * QTILE + 128 + 3 * QTILE], dt.bfloat16, tag="pt")

                # greedy-pack slots into PSUM group tiles of GROUP*QTILE f32
                budget = GROUP * QTILE
                g_start = 0
                while g_start < len(slots):
                    g_end = g_start
                    used = 0
                    while g_end < len(slots) and used + slots[g_end][3] <= budget:
                        used += slots[g_end][3]
                        g_end += 1
                    ST = stp.tile([128, budget], dt.float32, tag="st")
                    base = slots[g_start][2]
                    for kind, idx, scol, w in slots[g_start:g_end]:
                        pcol = scol - base
                        if kind == "loc":
                            b0 = loc[idx][0]
                            rr = qs if w == QTILE else qs[:, 128:QTILE]
                            nc.tensor.matmul(
                                ST[:, pcol:pcol + w],
                                lhsT=KT[:, b0 * BLK: b0 * BLK + 128],
                                rhs=rr, start=True, stop=True,
                            )
                        else:
                            b = cands[idx]
                            p = 64 * (b % 2)
                            nc.tensor.matmul(
                                ST[p:p + 64, pcol:pcol + w],
                                lhsT=KT[:, b * BLK:(b + 1) * BLK],
                                rhs=qs, start=True, stop=True,
                                tile_position=(0, p),
                            )
                    nc.scalar.activation(
                        PT[:, base: base + used], ST[:, :used], EXP, scale=SCALE,
                    )
                    g_start = g_end

                # causal masks on the diagonal chunks (D0 full, D1 half-width)
                scols = {(kk, ii): cc for kk, ii, cc, _w in slots}
                d0 = scols[("loc", nloc - 2)]
                nc.vector.tensor_mul(PT[:, d0:d0 + QTILE], PT[:, d0:d0 + QTILE], M1)
                d1 = scols[("loc", nloc - 1)]
                nc.vector.tensor_mul(PT[:, d1:d1 + 128], PT[:, d1:d1 + 128], M1[:, 0:128])

                # staircase mask over the two leading chunks (blocks lo..lo+3)
                if 4 * t - 16 >= 0:
                    mcol = (h * 4 + (t - 4)) * 512
                    nc.vector.tensor_mul(PT[:, 0:512], PT[:, 0:512], SMASK[:, mcol:mcol + 512])

                # remote candidates: keep only if this core's head selects them
                for kind, ri, scol, w in [sl for sl in slots if sl[0] == "rem"]:
                    b = cands[ri]
                    p = 64 * (b % 2)
                    col = h * VSTRIDE + (b % VSTRIDE)
                    ap = PT[p:p + 64, scol:scol + w]
                    nc.vector.tensor_scalar_mul(ap, ap, RSEL[p:p + 64, col:col + 1])

                return (h, t, loc, cands, slots, PT, OH)

            def emit_pv(ctxt):
                h, t, loc, cands, slots, PT, OH = ctxt
                nloc = len(loc)
                OP = opp.tile([128, 2 * 129], dt.float32, tag="op")
                for s in range(2):
                    o_ap = OP[:, s * 129:(s + 1) * 129]
                    # D1 (last local slot) has no s=0 half: fully masked there
                    mms = [sl for sl in slots if not (sl[0] == "loc" and sl[1] == nloc - 1 and s == 0)]
                    for mm, (kind, idx, scol, w) in enumerate(mms):
                        if kind == "loc":
                            b0 = loc[idx][0]
                            lcol = scol + s * 128 if w == QTILE else scol
                            nc.tensor.matmul(
                                o_ap,
                                lhsT=PT[:, lcol: lcol + 128],
                                rhs=V1[:, (b0 // 2) * 129: (b0 // 2) * 129 + 129],
                                start=(mm == 0), stop=(mm == len(mms) - 1),
                            )
                        else:
                            b = cands[idx]
                            p = 64 * (b % 2)
                            nc.tensor.matmul(
                                o_ap,
                                lhsT=PT[p:p + 64, scol + s * 128: scol + s * 128 + 128],
                                rhs=V1[p:p + 64, (b // 2) * 129: (b // 2) * 129 + 129],
                                start=(mm == 0), stop=(mm == len(mms) - 1),
                            )

                # normalize: O / L (L = ones-column result at col 128 of each half)
                LI = lp.tile([128, 2], dt.float32, tag="li")
                l_ap = OP[:, :].rearrange("p (s x) -> p s x", s=2)[:, :, 128]
                nc.vector.reciprocal(LI, l_ap)
                for s in range(2):
                    nc.vector.tensor_scalar_mul(
                        OH[:, 2 * t + s, :],
                        OP[:, s * 129: s * 129 + 128],
                        LI[:, s:s + 1],
                    )
                if h < HPC - 1:
                    store = {3: (0, 8), 5: (8, 12), 7: (12, 16)}.get(t)
                else:
                    store = {4: (8, 16), 1: (2, 8), 0: (0, 2)}.get(t)
                if store is not None:
                    c0, c1 = store
                    nc.sync.dma_start(out=out_r[:, c0:c1, h * 128:(h + 1) * 128],
                                      in_=OH[:, c0:c1, :])

            out_r = out.rearrange("(c p) m -> p c m", p=128)  # [128, 16, 512]
            prev = None
            for h in range(HPC):
                OH = ohp.tile([128, NCHUNK, 128], dt.float32, tag="oh")
                # last head runs t descending so the pipeline drains on the
                # smallest tile instead of the largest
                order = range(NT) if h < HPC - 1 else range(NT - 1, -1, -1)
                for t in order:
                    ctxt = emit_scores(h, t, OH)
                    if prev is not None:
                        emit_pv(prev)
                    prev = ctxt
            emit_pv(prev)

    nc.finalize()
    return nc


def _host_inputs(query, key, value):
    """Build the 8 per-core input maps (host-side sharding + layout prep)."""
    q = np.asarray(query, dtype=np.float32)
    k = np.asarray(key, dtype=np.float32)
    v = np.asarray(value, dtype=np.float32)

    pp = np.arange(128)[:, None]
    qq = np.arange(QTILE)[None, :]
    m1 = (qq >= pp).astype(BF16)

    in_maps = []
    for c in range(NCORES):
        qTc = np.ascontiguousarray(q[:, c * 512:(c + 1) * 512].T).astype(BF16)
        kTc = np.ascontiguousarray(k[:, c * D:(c + 1) * D].T).astype(BF16)
        vc = v[:, c * D:(c + 1) * D]                         # [2048, 128]
        vch = vc.reshape(NCHUNK, 128, D).transpose(1, 0, 2)  # [128, 16, 128]
        v1c = np.ones((128, NCHUNK, 129), dtype=np.float32)
        v1c[:, :, :128] = vch
        v1c = v1c.reshape(128, NCHUNK * 129).astype(BF16)

        rselc = np.zeros((128, HPC * VSTRIDE), dtype=np.float32)
        # staircase mask: [128, HPC*4*512], block lo+i visible to q rows
        # [0, 64*i) or fully if remote-selected
        smaskc = np.zeros((128, HPC, 4, 2, QTILE), dtype=np.float32)
        for j in range(HPC):
            r = _remote_class(c, j)
            rselc[:, j * VSTRIDE + r] = 1.0
            for t in range(4, 8):
                lo = 4 * t - 16
                for i in range(4):
                    b = lo + i
                    chunk, ph = i // 2, 64 * (i % 2)
                    if b % VSTRIDE == r:
                        smaskc[ph:ph + 64, j, t - 4, chunk, :] = 1.0
                    else:
                        smaskc[ph:ph + 64, j, t - 4, chunk, :64 * i] = 1.0
        smaskc = smaskc.reshape(128, HPC * 4 * 512).astype(BF16)

        in_maps.append({
            "qT": qTc,
            "kT": kTc,
            "v1": v1c,
            "m1": m1,
            "smask": smaskc,
            "rsel": rselc,
        })
    return in_maps


def _get_nc():
    if "nc" not in _CACHE:
        _CACHE["nc"] = _build_nc()
    return _CACHE["nc"]


def kernel(query, key, value):
    from concourse.bass_utils import run_bass_kernel_spmd

    nc = _get_nc()
    in_maps = _host_inputs(query, key, value)
    res = run_bass_kernel_spmd(nc, in_maps, core_ids=list(range(NCORES)))
    outs = [res.results[c]["out"] for c in range(NCORES)]
    return np.concatenate(outs, axis=1).astype(np.float32)


if __name__ == "__main__":
    rng = np.random.default_rng(0)
    q = rng.standard_normal((S, NUM_HEADS * D), dtype=np.float32)
    k = rng.standard_normal((S, NUM_KV_HEADS * D), dtype=np.float32)
    v = rng.standard_normal((S, NUM_KV_HEADS * D), dtype=np.float32)
    o = kernel(query=q, key=k, value=v)
    print("kernel output", o.shape, o.dtype, np.abs(o).max())
